# revision 20
# baseline (speedup 1.0000x reference)
"""Trainium2 Bass kernel for nn_EnokeeEncoder (segment_reduce).

Reference semantics:
    lhs = embed[input_ids]                      # only lhs[:, :32, :] is ever used
    m[b,j,x] = (pos[b,j,x] != -1) & (am[b,j] != 0)
    pooled = einsum('bml,bld->bmd', m, lhs[:, :32]) / 32
    x = LayerNorm(pooled) * gamma + beta
    out = (x @ w1) @ w2 + b2                    # [16, 64, 100000]

Device strategy (8 cores, SPMD, no collectives):
  - mention rows with an all-zero mask (am==0 or empty prefix) produce the
    constant row (beta @ w1) @ w2 + b2 — filled on the host. Active
    mentions are compacted; the device computes floor(n_act/128) full
    128-token tiles, the sub-tile remainder (<128 rows) is computed on the
    host in fp32 (bounded: <1/4 of a percent of total flops per row).
  - everything upstream of the classifier is folded on the host into three
    tiny per-batch tensors (pooling is linear in the mask):
        yT   = (emb @ (gamma.w1)).T @ m      via ew1   [128, 4, 128]
        mu   = mean_d pooled                 via esum  [128, 4]   (1/D folded)
        e2   = mean_d pooled^2 = m.T G m     via Gram  gg [128, 4, 128]
    so the device does 3 small matmuls per group + a short LN tail.
  - hT = rs*yT + u*(-mu*rs) + c is assembled with two outer-product
    PSUM folds (P1 = u (x) nmurs + c (x) 1, P2 = 1 (x) rs).
  - output projection is tensor-parallel over the entity vocab:
    core c computes out[:, c*12500:(c+1)*12500] = hT.T @ w2[:, shard].
  - w2 / hT / output are bf16 (tolerance 2e-2, bf16 contributes ~4e-3).
  - prework inputs arrive as ONE packed [128, *] bf16 DMA; w2 as 5 column
    tiles on the ACT ring; all output DMAs ride the otherwise-idle SWDGE
    (gpsimd) ring so the sync/ACT queues stay clean.
  - main-loop PSUM is all pairs [128, 2, 512] (bufs=4 = 8 banks); one
    DVE/ACT instruction evacuates two 500-col chunks.
"""

import sys

if '/opt/trn_rl_repo' not in sys.path:
    sys.path.insert(0, '/opt/trn_rl_repo')

import numpy as np
import ml_dtypes

import concourse.bass as bass
import concourse.mybir as mybir
import concourse.tile as tile
from concourse import bacc
from concourse.bass_utils import run_bass_kernel_spmd

# model dims (fixed by the problem)
B, S, M, L, D = 16, 512, 64, 32, 1024
V, R, E = 32000, 128, 100000
LN_EPS = 1e-5

N_CORES = 8
ES = E // N_CORES      # 12500 entity columns per core
ECH = 500              # main-matmul moving chunk (<=512 fp32 psum)
NEC = ES // ECH        # 25 chunks
NW2 = 5                # w2 arrives as 5 column tiles of 2500
W2C = ES // NW2

F32 = mybir.dt.float32
F32R = mybir.dt.float32r
BF16 = mybir.dt.bfloat16
AF = mybir.AluOpType
ACTF = mybir.ActivationFunctionType
BF16NP = ml_dtypes.bfloat16


def _bank_segs(a, b):
    """Split [a, b) at 512-column PSUM bank boundaries."""
    segs = []
    while a < b:
        nxt = min(b, (a // 512 + 1) * 512)
        segs.append((a, nxt))
        a = nxt
    return segs


def build_nc(has_b2: bool, dwidths: tuple):
    """dwidths = device-token count per batch-group (sum divisible by 128)."""
    offs = [0]
    for w in dwidths:
        offs.append(offs[-1] + w)
    TP = offs[4]
    assert TP % 128 == 0 and TP > 0
    TT = TP // 128
    # packed prework tensor layout (bf16): [mask TP | ew1 512 | gg 512 | esum 4]
    PK_EW1, PK_GG, PK_ES = TP, TP + 512, TP + 1024
    PK = TP + 1028
    PKP = ((PK + 15) // 16) * 16       # xbar needs row count % 16 == 0
    print(f"[kernel] build_nc: has_b2={has_b2} dwidths={dwidths} TP={TP}",
          flush=True)

    nc = bacc.Bacc("TRN2", target_bir_lowering=False, debug=False,
                   enable_asserts=False, num_devices=N_CORES)

    # ---- DRAM I/O (per-core) ----
    d_pk = nc.dram_tensor("packedT", [PKP, 128], BF16, kind="ExternalInput").ap()
    d_curow = nc.dram_tensor("curow", [1, 256], F32, kind="ExternalInput").ap()
    d_onesr = nc.dram_tensor("onesr", [1, 128], F32, kind="ExternalInput").ap()
    d_w2 = nc.dram_tensor("w2s", [R, ES], BF16, kind="ExternalInput").ap()
    d_b2 = nc.dram_tensor("b2s", [1, ES], F32, kind="ExternalInput").ap()
    d_out = nc.dram_tensor("out", [TP, ES], BF16, kind="ExternalOutput").ap()

    def tchunks(step=256):
        return [slice(t0, min(t0 + step, TP)) for t0 in range(0, TP, step)]

    with tile.TileContext(nc) as tc:
        with (
            tc.tile_pool(name="persist", bufs=1) as pp,
            tc.tile_pool(name="pre", bufs=1) as pre,
        ):
            hT_sb = pp.tile([R, TP], BF16)
            w2t = [pp.tile([R, W2C], BF16, name=f"w2t{i}") for i in range(NW2)]
            for i in range(NW2):
                nc.scalar.dma_start(w2t[i][:], d_w2[:, i * W2C:(i + 1) * W2C])

            pk_sb = pre.tile([128, PKP], BF16)
            nc.sync.dma_start_transpose(pk_sb[:], d_pk[:])
            curow_sb = pre.tile([1, 256], F32)
            nc.sync.dma_start(curow_sb[:], d_curow[:])
            onesr_sb = pre.tile([1, 128], F32)
            nc.sync.dma_start(onesr_sb[:], d_onesr[:])

            mask_ap = pk_sb[:, 0:TP]

            def ew1_ap(g):
                return pk_sb[:, PK_EW1 + g * 128:PK_EW1 + (g + 1) * 128]

            def gg_ap(g):
                return pk_sb[:, PK_GG + g * 128:PK_GG + (g + 1) * 128]

            def esum_ap(g):
                return pk_sb[:, PK_ES + g:PK_ES + g + 1]

            # ACT rsqrt-table preload while DMAs land
            dum_sb = pre.tile([1, 16], F32)
            nc.vector.memset(dum_sb[:], 1.0)
            nc.scalar.activation(dum_sb[:], dum_sb[:], ACTF.Abs_reciprocal_sqrt)

            onesrr_sb = pre.tile([1, 128], F32R)
            nc.vector.tensor_copy(onesrr_sb[:], onesr_sb[:])
            onesbf_sb = pre.tile([128, 1], BF16)
            nc.vector.memset(onesbf_sb[:], 1.0)
            curowr_sb = pre.tile([1, 256], F32R)
            nc.vector.tensor_copy(curowr_sb[:], curow_sb[:])
            onestp0_sb = pre.tile([1, TP], F32)
            nc.vector.memset(onestp0_sb[:], 1.0)
            onestp_sb = pre.tile([1, TP], F32R)
            nc.vector.tensor_copy(onestp_sb[:], onestp0_sb[:])

            # ---- per-group folds: yT, mu(s1), qm — 3 matmuls per group ----
            fold_cm = tc.tile_pool(name="foldps", bufs=1, space="PSUM")
            fps = fold_cm.__enter__()
            yT_ps = fps.tile([128, TP], F32)
            with tc.tile_pool(name="qmps", bufs=1, space="PSUM") as qps:
                s1_ps = qps.tile([1, TP], F32)
                e2_ps = qps.tile([1, TP], F32)
                qm_ps = qps.tile([128, TP], F32)
                for g in range(4):
                    if dwidths[g] == 0:
                        continue
                    for a, b in _bank_segs(offs[g], offs[g + 1]):
                        sl = slice(a, b)
                        nc.tensor.matmul(out=yT_ps[:, sl], lhsT=ew1_ap(g),
                                         rhs=mask_ap[:, sl],
                                         start=True, stop=True,
                                         skip_group_check=True)
                        nc.tensor.matmul(out=qm_ps[:, sl], lhsT=gg_ap(g),
                                         rhs=mask_ap[:, sl],
                                         start=True, stop=True,
                                         skip_group_check=True)
                        nc.tensor.matmul(out=s1_ps[:, sl], lhsT=esum_ap(g),
                                         rhs=mask_ap[:, sl],
                                         start=True, stop=True,
                                         skip_group_check=True)

                # musq off the critical chain (needs only s1)
                musq_sb = pre.tile([1, TP], F32R)
                for ch in tchunks(256):
                    nc.scalar.square(musq_sb[:, ch], s1_ps[:, ch])
                # yT evacuated early so t1 has a single-PSUM input later
                yTs_sb = pre.tile([128, TP], F32)
                for ch in tchunks(256):
                    nc.vector.tensor_copy(yTs_sb[:, ch], yT_ps[:, ch])
                # mq = qm (.) mask ; e2 = colsum(mq)   (quadratic form)
                mq_sb = pre.tile([128, TP], BF16)
                for ch in tchunks(512):
                    nc.vector.tensor_tensor(mq_sb[:, ch], qm_ps[:, ch],
                                            mask_ap[:, ch], op=AF.mult)
                    nc.tensor.matmul(out=e2_ps[:, ch], lhsT=onesbf_sb[:],
                                     rhs=mq_sb[:, ch], start=True, stop=True,
                                     skip_group_check=True)

                # ---- LN tail (1/D already folded into esum/gram) ----
                # vare = (e2 + eps) - musq ; rs = rsqrt(vare) ; nmurs = -mu*rs
                vare_sb = pre.tile([1, TP], F32R)
                rs_sb = pre.tile([1, TP], F32R)
                nmurs_sb = pre.tile([1, TP], F32R)
                for ch in tchunks(256):
                    nc.vector.scalar_tensor_tensor(vare_sb[:, ch],
                                                   in0=e2_ps[:, ch],
                                                   scalar=LN_EPS,
                                                   in1=musq_sb[:, ch],
                                                   op0=AF.add,
                                                   op1=AF.subtract)
                    nc.scalar.activation(rs_sb[:, ch], vare_sb[:, ch],
                                         ACTF.Abs_reciprocal_sqrt)
                    nc.vector.scalar_tensor_tensor(nmurs_sb[:, ch],
                                                   in0=s1_ps[:, ch],
                                                   scalar=-1.0,
                                                   in1=rs_sb[:, ch],
                                                   op0=AF.mult, op1=AF.mult)

            with tc.tile_pool(name="bcps", bufs=1, space="PSUM") as bps:
                # P2 = 1 (x) rs ; P1 = u (x) nmurs + c (x) 1 ; then
                # hT = yT*P2 + P1, emitted per 256-chunk so the main loop
                # can start on early token tiles.
                p2_ps = bps.tile([128, TP], F32)
                p1_ps = bps.tile([128, TP], F32)
                t1_sb = pre.tile([128, TP], F32)
                for ch in tchunks(256):
                    nc.tensor.matmul(out=p2_ps[:, ch], lhsT=onesrr_sb[:],
                                     rhs=rs_sb[:, ch], start=True,
                                     stop=True, skip_group_check=True)
                    nc.tensor.matmul(out=p1_ps[:, ch],
                                     lhsT=curowr_sb[:, 128:256],
                                     rhs=nmurs_sb[:, ch], start=True,
                                     stop=False, skip_group_check=True)
                    nc.tensor.matmul(out=p1_ps[:, ch],
                                     lhsT=curowr_sb[:, 0:128],
                                     rhs=onestp_sb[:, ch], start=False,
                                     stop=True, skip_group_check=True)
                    nc.vector.tensor_tensor(t1_sb[:, ch], p2_ps[:, ch],
                                            yTs_sb[:, ch], op=AF.mult)
                    nc.vector.tensor_tensor(hT_sb[:, ch], t1_sb[:, ch],
                                            p1_ps[:, ch], op=AF.add)
            fold_cm.__exit__(None, None, None)

            # ---- main: out[t, e] = hT.T @ w2 (+ b2), bf16 out ----
            NPAIR = (NEC + 1) // 2     # 13 pair-slots (last holds 1 chunk)
            HCOLS = 12 * ECH           # 6000: first-half DMA boundary

            def w2ap(ec):
                i, j = divmod(ec * ECH, W2C)
                return w2t[i][:, j:j + ECH]

            if has_b2:
                with tc.tile_pool(name="b2p", bufs=1) as b2p, \
                     tc.tile_pool(name="b2ps", bufs=2, space="PSUM") as bpp:
                    b2c = b2p.tile([1, ES], F32)
                    nc.sync.dma_start(b2c[:], d_b2[:])
                    b2cr = b2p.tile([1, ES], F32R)
                    nc.vector.tensor_copy(b2cr[:], b2c[:])
                    bb_sb = pre.tile([128, ES], F32)
                    for ec in range(NEC):
                        esl = slice(ec * ECH, (ec + 1) * ECH)
                        bb_ps = bpp.tile([128, ECH], F32, tag="bbp")
                        nc.tensor.matmul(out=bb_ps[:], lhsT=onesrr_sb[:],
                                         rhs=b2cr[:, esl], start=True,
                                         stop=True)
                        nc.vector.tensor_copy(bb_sb[:, esl], bb_ps[:])

            with tc.tile_pool(name="mm2", bufs=4, space="PSUM") as mp2, \
                 tc.tile_pool(name="outp", bufs=3) as op:
                for tt in range(TT):
                    lhs = hT_sb[:, tt * 128:(tt + 1) * 128]
                    trow = slice(tt * 128, (tt + 1) * 128)
                    o_sb = op.tile([128, ES], BF16, tag="o")
                    for p in range(NPAIR):
                        c0, c1 = 2 * p, 2 * p + 1
                        pt = mp2.tile([128, 2, 512], F32, tag="mm2")
                        nc.tensor.matmul(out=pt[:, 0, 0:ECH], lhsT=lhs,
                                         rhs=w2ap(c0), start=True, stop=True)
                        if c1 < NEC:
                            nc.tensor.matmul(out=pt[:, 1, 0:ECH], lhsT=lhs,
                                             rhs=w2ap(c1), start=True,
                                             stop=True)
                        even = (tt * NPAIR + p) % 2 == 0
                        if c1 < NEC:
                            osl = o_sb[:, c0 * ECH:(c1 + 1) * ECH]
                            dst = osl.rearrange("q (two c) -> q two c", two=2)
                            src = pt[:, :, 0:ECH]
                        else:
                            dst = o_sb[:, c0 * ECH:(c0 + 1) * ECH]
                            src = pt[:, 0, 0:ECH]
                        if has_b2:
                            if c1 < NEC:
                                bsl = bb_sb[:, c0 * ECH:(c1 + 1) * ECH]
                                bsl = bsl.rearrange("q (two c) -> q two c",
                                                    two=2)
                            else:
                                bsl = bb_sb[:, c0 * ECH:(c0 + 1) * ECH]
                            nc.vector.tensor_tensor(dst, src, bsl, op=AF.add)
                        elif even:
                            nc.vector.tensor_copy(dst, src)
                        else:
                            nc.scalar.copy(dst, src)
                        if p == 2:
                            nc.gpsimd.dma_start(d_out[trow, 0:3000],
                                                o_sb[:, 0:3000])
                        elif p == 5:
                            nc.gpsimd.dma_start(d_out[trow, 3000:HCOLS],
                                                o_sb[:, 3000:HCOLS])
                    nc.gpsimd.dma_start(d_out[trow, HCOLS:ES],
                                        o_sb[:, HCOLS:ES])

    nc.finalize()
    return nc


_NC_CACHE = {}


def _get_nc(has_b2: bool, dwidths: tuple):
    key = (has_b2, dwidths)
    if key not in _NC_CACHE:
        _NC_CACHE[key] = build_nc(has_b2, dwidths)
    return _NC_CACHE[key]


def prep_core_inputs(inputs):
    """Host-side folds + compaction. Returns (shared, w2s, b2s, meta)."""
    ids = np.asarray(inputs["input_ids"]).astype(np.int64)[:, :L]      # [16, 32]
    pos = np.asarray(inputs["entity_position_ids"])                    # [B, M, L]
    am = np.asarray(inputs["entity_attention_mask"])                   # [B, M]
    embed = np.asarray(inputs["embed"], dtype=np.float32)
    gamma = np.asarray(inputs["ln_gamma"], dtype=np.float32)
    beta = np.asarray(inputs["ln_beta"], dtype=np.float32)
    w1 = np.asarray(inputs["w1"], dtype=np.float32)
    w2 = np.asarray(inputs["w2"], dtype=np.float32)
    b2 = np.asarray(inputs["b2"], dtype=np.float32)

    mrow = (pos != -1)                                                 # [B, M, L]
    active = (am != 0) & mrow.any(-1)                                  # [B, M]

    emb = embed[ids]                                                   # [B, 32, D]
    w1g = w1 * gamma[:, None]                                          # [D, R]
    ew1 = emb @ w1g                                                    # [B, 32, R]
    # 1/D folded: s1 matmul yields mu, gram quadratic form yields E[x^2]
    gram = np.einsum('bxd,byd->bxy', emb, emb) / np.float32(D)         # [B,32,32]
    esum = emb.sum(-1) / np.float32(D)                                 # [B, 32]
    c_row = beta @ w1                                                  # [R]
    u_row = gamma @ w1                                                 # [R]

    # group g holds batches 4g..4g+3 on partitions 32k..32k+32 (k = b-4g);
    # groups occupy consecutive column ranges of variable width.
    tok_lists = []
    for g in range(4):
        toks = []
        for k in range(4):
            b = 4 * g + k
            for j in np.nonzero(active[b])[0]:
                toks.append((k, b, int(j)))
        tok_lists.append(toks)
    n_act = sum(len(t) for t in tok_lists)
    TP = (n_act // 128) * 128          # device tokens; remainder on host
    # clip the flat token list at TP to get device widths
    dwidths, acc = [], 0
    for g in range(4):
        take = max(0, min(len(tok_lists[g]), TP - acc))
        dwidths.append(take)
        acc += take
    dwidths = tuple(dwidths)

    tok_flat = [t for g in range(4) for t in tok_lists[g]]
    tok_idx = np.asarray([b * M + j for (_, b, j) in tok_flat], np.int64)

    PK_EW1, PK_GG, PK_ES = TP, TP + 512, TP + 1024
    PK = TP + 1028
    PKP = ((PK + 15) // 16) * 16
    packed = np.zeros((128, PKP), np.float32)
    for col, (k, b, j) in enumerate(tok_flat[:TP]):
        packed[32 * k:32 * k + 32, col] = mrow[b, j] / np.float32(L)
    for g in range(4):
        for k in range(4):
            b = 4 * g + k
            rows = slice(32 * k, 32 * k + 32)
            packed[rows, PK_EW1 + g * 128:PK_EW1 + (g + 1) * 128] = ew1[b]
            packed[rows, PK_GG + g * 128 + 32 * k:
                   PK_GG + g * 128 + 32 * k + 32] = gram[b]
            packed[rows, PK_ES + g] = esum[b]
    curow = np.concatenate([c_row, u_row]).reshape(1, 2 * R).astype(np.float32)

    shared = {
        "packedT": np.ascontiguousarray(packed.astype(BF16NP).T),
        "curow": curow,
        "onesr": np.ones((1, 128), np.float32),
    }
    w2s = [np.ascontiguousarray(w2[:, c * ES:(c + 1) * ES]).astype(BF16NP)
           for c in range(N_CORES)]
    b2s = [np.ascontiguousarray(b2[c * ES:(c + 1) * ES].reshape(1, ES))
           for c in range(N_CORES)]
    has_b2 = bool(np.any(b2 != 0.0))
    const_row = (beta @ w1) @ w2 + b2                                  # [E]

    # host-side fp32 rows for the remainder tokens (and the TTd==0 case)
    rem_rows = None
    if n_act > TP:
        rsel = tok_flat[TP:]
        bs = np.asarray([b for (_, b, _) in rsel])
        js = np.asarray([j for (_, _, j) in rsel])
        mr = mrow[bs, js].astype(np.float32) / np.float32(L)           # [nr, 32]
        pooled_r = np.einsum('rx,rxd->rd', mr, emb[bs])                # [nr, D]
        mu = pooled_r.mean(-1, keepdims=True)
        var = ((pooled_r - mu) ** 2).mean(-1, keepdims=True)
        x = (pooled_r - mu) / np.sqrt(var + LN_EPS) * gamma + beta
        rem_rows = (x @ w1) @ w2 + b2                                  # [nr, E]

    meta = {
        "has_b2": has_b2,
        "dwidths": dwidths,
        "TP": TP,
        "tok_idx": tok_idx,
        "const_row": const_row.astype(np.float32),
        "rem_rows": rem_rows,
        "active": active,
    }
    return shared, w2s, b2s, meta


def _bf16_to_f32(a):
    return (a.view(np.uint16).astype(np.uint32) << 16).view(np.float32)


def kernel(**inputs) -> np.ndarray:
    shared, w2s, b2s, meta = prep_core_inputs(inputs)
    TP = meta["TP"]
    full = np.zeros((B * M, E), np.float32)
    tok_idx = meta["tok_idx"]
    if TP > 0:
        nc = _get_nc(meta["has_b2"], meta["dwidths"])
        in_maps = [dict(shared, w2s=w2s[c], b2s=b2s[c])
                   for c in range(N_CORES)]
        res = run_bass_kernel_spmd(nc, in_maps, list(range(N_CORES)))
        buf = np.empty((TP, E), np.float32)
        for c in range(N_CORES):
            blk = np.asarray(res.results[c]["out"])          # bf16 [TP, ES]
            buf[:, c * ES:(c + 1) * ES] = _bf16_to_f32(
                np.ascontiguousarray(blk))
        full[tok_idx[:TP]] = buf
    if meta["rem_rows"] is not None:
        full[tok_idx[TP:]] = meta["rem_rows"]
    cr = meta["const_row"]
    if np.any(cr != 0.0):
        inactive = np.nonzero(~meta["active"].reshape(-1))[0]
        full[inactive] = cr
    return np.ascontiguousarray(full.reshape(B, M, E))


# revision 21
# speedup vs baseline: 1.1941x; 1.1941x over previous
"""Trainium2 Bass kernel for nn_EnokeeEncoder (segment_reduce).

Reference semantics:
    lhs = embed[input_ids]                      # only lhs[:, :32, :] is ever used
    m[b,j,x] = (pos[b,j,x] != -1) & (am[b,j] != 0)
    pooled = einsum('bml,bld->bmd', m, lhs[:, :32]) / 32
    x = LayerNorm(pooled) * gamma + beta
    out = (x @ w1) @ w2 + b2                    # [16, 64, 100000]

Device strategy (8 cores, SPMD, no collectives):
  - mention rows with an all-zero mask (am==0 or empty prefix) produce the
    constant row (beta @ w1) @ w2 + b2 — filled on the host. Active
    mentions are compacted; the device computes floor(n_act/128) full
    128-token tiles, the sub-tile remainder (<128 rows) is computed on the
    host in fp32 (bounded: <1/4 of a percent of total flops per row).
  - everything upstream of the classifier is folded on the host into three
    tiny per-batch tensors (pooling is linear in the mask):
        yT   = (emb @ (gamma.w1)).T @ m      via ew1   [128, 4, 128]
        mu   = mean_d pooled                 via esum  [128, 4]   (1/D folded)
        e2   = mean_d pooled^2 = m.T G m     via Gram  gg [128, 4, 128]
    so the device does 3 small matmuls per group + a short LN tail.
  - hT = rs*yT + u*(-mu*rs) + c is assembled with two outer-product
    PSUM folds (P1 = u (x) nmurs + c (x) 1, P2 = 1 (x) rs).
  - output projection is tensor-parallel over the entity vocab:
    core c computes out[:, c*12500:(c+1)*12500] = hT.T @ w2[:, shard].
  - w2 / hT / output are bf16 (tolerance 2e-2, bf16 contributes ~4e-3).
  - prework inputs arrive as ONE packed [128, *] bf16 DMA; w2 as 5 column
    tiles on the ACT ring; all output DMAs ride the otherwise-idle SWDGE
    (gpsimd) ring so the sync/ACT queues stay clean.
  - main-loop PSUM is all pairs [128, 2, 512] (bufs=4 = 8 banks); one
    DVE/ACT instruction evacuates two 500-col chunks.
"""

import sys

if '/opt/trn_rl_repo' not in sys.path:
    sys.path.insert(0, '/opt/trn_rl_repo')

import numpy as np
import ml_dtypes

import concourse.bass as bass
import concourse.mybir as mybir
import concourse.tile as tile
from concourse import bacc
from concourse.bass_utils import run_bass_kernel_spmd

# model dims (fixed by the problem)
B, S, M, L, D = 16, 512, 64, 32, 1024
V, R, E = 32000, 128, 100000
LN_EPS = 1e-5

N_CORES = 8
ES = E // N_CORES      # 12500 entity columns per core
ECH = 500              # main-matmul moving chunk (<=512 fp32 psum)
NEC = ES // ECH        # 25 chunks
NW2 = 5                # w2 arrives as 5 column tiles of 2500
W2C = ES // NW2

F32 = mybir.dt.float32
F32R = mybir.dt.float32r
BF16 = mybir.dt.bfloat16
AF = mybir.AluOpType
ACTF = mybir.ActivationFunctionType
BF16NP = ml_dtypes.bfloat16


def _bank_segs(a, b):
    """Split [a, b) at 512-column PSUM bank boundaries."""
    segs = []
    while a < b:
        nxt = min(b, (a // 512 + 1) * 512)
        segs.append((a, nxt))
        a = nxt
    return segs


def build_nc(has_b2: bool, dwidths: tuple):
    """dwidths = device-token count per batch-group (sum divisible by 128)."""
    offs = [0]
    for w in dwidths:
        offs.append(offs[-1] + w)
    TP = offs[4]
    assert TP % 128 == 0 and TP > 0
    TT = TP // 128
    # packed prework tensor layout (bf16): [mask TP | ew1 512 | gg 512 | esum 4]
    PK_EW1, PK_GG, PK_ES = TP, TP + 512, TP + 1024
    PK = TP + 1028
    PKP = ((PK + 15) // 16) * 16       # xbar needs row count % 16 == 0
    print(f"[kernel] build_nc: has_b2={has_b2} dwidths={dwidths} TP={TP}",
          flush=True)

    nc = bacc.Bacc("TRN2", target_bir_lowering=False, debug=False,
                   enable_asserts=False, num_devices=N_CORES)

    # ---- DRAM I/O (per-core) ----
    d_pk = nc.dram_tensor("packed", [128, PKP], BF16, kind="ExternalInput").ap()
    d_curow = nc.dram_tensor("curow", [1, 256], F32, kind="ExternalInput").ap()
    d_onesr = nc.dram_tensor("onesr", [1, 128], F32, kind="ExternalInput").ap()
    d_w2 = nc.dram_tensor("w2s", [R, ES], BF16, kind="ExternalInput").ap()
    d_b2 = nc.dram_tensor("b2s", [1, ES], F32, kind="ExternalInput").ap()
    d_out = nc.dram_tensor("out", [TP, ES], BF16, kind="ExternalOutput").ap()

    def tchunks(step=256):
        return [slice(t0, min(t0 + step, TP)) for t0 in range(0, TP, step)]

    with tile.TileContext(nc) as tc:
        with (
            tc.tile_pool(name="persist", bufs=1) as pp,
            tc.tile_pool(name="pre", bufs=1) as pre,
        ):
            hT_sb = pp.tile([R, TP], BF16)
            w2t = [pp.tile([R, W2C], BF16, name=f"w2t{i}") for i in range(NW2)]
            for i in range(NW2):
                nc.scalar.dma_start(w2t[i][:], d_w2[:, i * W2C:(i + 1) * W2C])

            pk_sb = pre.tile([128, PKP], BF16)
            nc.sync.dma_start(pk_sb[:], d_pk[:])
            curow_sb = pre.tile([1, 256], F32)
            nc.sync.dma_start(curow_sb[:], d_curow[:])
            onesr_sb = pre.tile([1, 128], F32)
            nc.sync.dma_start(onesr_sb[:], d_onesr[:])

            mask_ap = pk_sb[:, 0:TP]

            def ew1_ap(g):
                return pk_sb[:, PK_EW1 + g * 128:PK_EW1 + (g + 1) * 128]

            def gg_ap(g):
                return pk_sb[:, PK_GG + g * 128:PK_GG + (g + 1) * 128]

            def esum_ap(g):
                return pk_sb[:, PK_ES + g:PK_ES + g + 1]

            # ACT rsqrt-table preload while DMAs land
            dum_sb = pre.tile([1, 16], F32)
            nc.vector.memset(dum_sb[:], 1.0)
            nc.scalar.activation(dum_sb[:], dum_sb[:], ACTF.Abs_reciprocal_sqrt)

            onesrr_sb = pre.tile([1, 128], F32R)
            nc.vector.tensor_copy(onesrr_sb[:], onesr_sb[:])
            onesbf_sb = pre.tile([128, 1], BF16)
            nc.vector.memset(onesbf_sb[:], 1.0)
            curowr_sb = pre.tile([1, 256], F32R)
            nc.vector.tensor_copy(curowr_sb[:], curow_sb[:])
            onestp0_sb = pre.tile([1, TP], F32)
            nc.vector.memset(onestp0_sb[:], 1.0)
            onestp_sb = pre.tile([1, TP], F32R)
            nc.vector.tensor_copy(onestp_sb[:], onestp0_sb[:])

            # ---- per-group folds: yT, mu(s1), qm — 3 matmuls per group ----
            fold_cm = tc.tile_pool(name="foldps", bufs=1, space="PSUM")
            fps = fold_cm.__enter__()
            yT_ps = fps.tile([128, TP], F32)
            with tc.tile_pool(name="qmps", bufs=1, space="PSUM") as qps:
                s1_ps = qps.tile([1, TP], F32)
                e2_ps = qps.tile([1, TP], F32)
                qm_ps = qps.tile([128, TP], F32)
                for g in range(4):
                    if dwidths[g] == 0:
                        continue
                    for a, b in _bank_segs(offs[g], offs[g + 1]):
                        sl = slice(a, b)
                        nc.tensor.matmul(out=yT_ps[:, sl], lhsT=ew1_ap(g),
                                         rhs=mask_ap[:, sl],
                                         start=True, stop=True,
                                         skip_group_check=True)
                        nc.tensor.matmul(out=qm_ps[:, sl], lhsT=gg_ap(g),
                                         rhs=mask_ap[:, sl],
                                         start=True, stop=True,
                                         skip_group_check=True)
                        nc.tensor.matmul(out=s1_ps[:, sl], lhsT=esum_ap(g),
                                         rhs=mask_ap[:, sl],
                                         start=True, stop=True,
                                         skip_group_check=True)

                # musq off the critical chain (needs only s1)
                musq_sb = pre.tile([1, TP], F32R)
                for ch in tchunks(256):
                    nc.scalar.square(musq_sb[:, ch], s1_ps[:, ch])
                # yT evacuated early so t1 has a single-PSUM input later
                yTs_sb = pre.tile([128, TP], F32)
                for ch in tchunks(256):
                    nc.vector.tensor_copy(yTs_sb[:, ch], yT_ps[:, ch])
                # mq = qm (.) mask ; e2 = colsum(mq)   (quadratic form)
                mq_sb = pre.tile([128, TP], BF16)
                for ch in tchunks(512):
                    nc.vector.tensor_tensor(mq_sb[:, ch], qm_ps[:, ch],
                                            mask_ap[:, ch], op=AF.mult)
                    nc.tensor.matmul(out=e2_ps[:, ch], lhsT=onesbf_sb[:],
                                     rhs=mq_sb[:, ch], start=True, stop=True,
                                     skip_group_check=True)

                # ---- LN tail (1/D already folded into esum/gram) ----
                # vare = (e2 + eps) - musq ; rs = rsqrt(vare) ; nmurs = -mu*rs
                vare_sb = pre.tile([1, TP], F32R)
                rs_sb = pre.tile([1, TP], F32R)
                nmurs_sb = pre.tile([1, TP], F32R)
                for ch in tchunks(256):
                    nc.vector.scalar_tensor_tensor(vare_sb[:, ch],
                                                   in0=e2_ps[:, ch],
                                                   scalar=LN_EPS,
                                                   in1=musq_sb[:, ch],
                                                   op0=AF.add,
                                                   op1=AF.subtract)
                    nc.scalar.activation(rs_sb[:, ch], vare_sb[:, ch],
                                         ACTF.Abs_reciprocal_sqrt)
                    nc.vector.scalar_tensor_tensor(nmurs_sb[:, ch],
                                                   in0=s1_ps[:, ch],
                                                   scalar=-1.0,
                                                   in1=rs_sb[:, ch],
                                                   op0=AF.mult, op1=AF.mult)

            with tc.tile_pool(name="bcps", bufs=1, space="PSUM") as bps:
                # P2 = 1 (x) rs ; P1 = u (x) nmurs + c (x) 1 ; then
                # hT = yT*P2 + P1, emitted per 256-chunk so the main loop
                # can start on early token tiles.
                p2_ps = bps.tile([128, TP], F32)
                p1_ps = bps.tile([128, TP], F32)
                t1_sb = pre.tile([128, TP], F32)
                for ch in tchunks(256):
                    nc.tensor.matmul(out=p2_ps[:, ch], lhsT=onesrr_sb[:],
                                     rhs=rs_sb[:, ch], start=True,
                                     stop=True, skip_group_check=True)
                    nc.tensor.matmul(out=p1_ps[:, ch],
                                     lhsT=curowr_sb[:, 128:256],
                                     rhs=nmurs_sb[:, ch], start=True,
                                     stop=False, skip_group_check=True)
                    nc.tensor.matmul(out=p1_ps[:, ch],
                                     lhsT=curowr_sb[:, 0:128],
                                     rhs=onestp_sb[:, ch], start=False,
                                     stop=True, skip_group_check=True)
                    nc.vector.tensor_tensor(t1_sb[:, ch], p2_ps[:, ch],
                                            yTs_sb[:, ch], op=AF.mult)
                    nc.vector.tensor_tensor(hT_sb[:, ch], t1_sb[:, ch],
                                            p1_ps[:, ch], op=AF.add)
            fold_cm.__exit__(None, None, None)

            # ---- main: out[t, e] = hT.T @ w2 (+ b2), bf16 out ----
            NPAIR = (NEC + 1) // 2     # 13 pair-slots (last holds 1 chunk)
            HCOLS = 12 * ECH           # 6000: first-half DMA boundary

            def w2ap(ec):
                i, j = divmod(ec * ECH, W2C)
                return w2t[i][:, j:j + ECH]

            if has_b2:
                with tc.tile_pool(name="b2p", bufs=1) as b2p, \
                     tc.tile_pool(name="b2ps", bufs=2, space="PSUM") as bpp:
                    b2c = b2p.tile([1, ES], F32)
                    nc.sync.dma_start(b2c[:], d_b2[:])
                    b2cr = b2p.tile([1, ES], F32R)
                    nc.vector.tensor_copy(b2cr[:], b2c[:])
                    bb_sb = pre.tile([128, ES], F32)
                    for ec in range(NEC):
                        esl = slice(ec * ECH, (ec + 1) * ECH)
                        bb_ps = bpp.tile([128, ECH], F32, tag="bbp")
                        nc.tensor.matmul(out=bb_ps[:], lhsT=onesrr_sb[:],
                                         rhs=b2cr[:, esl], start=True,
                                         stop=True)
                        nc.vector.tensor_copy(bb_sb[:, esl], bb_ps[:])

            with tc.tile_pool(name="mm2", bufs=4, space="PSUM") as mp2, \
                 tc.tile_pool(name="outp", bufs=3) as op:
                for tt in range(TT):
                    lhs = hT_sb[:, tt * 128:(tt + 1) * 128]
                    trow = slice(tt * 128, (tt + 1) * 128)
                    o_sb = op.tile([128, ES], BF16, tag="o")
                    for p in range(NPAIR):
                        c0, c1 = 2 * p, 2 * p + 1
                        pt = mp2.tile([128, 2, 512], F32, tag="mm2")
                        nc.tensor.matmul(out=pt[:, 0, 0:ECH], lhsT=lhs,
                                         rhs=w2ap(c0), start=True, stop=True)
                        if c1 < NEC:
                            nc.tensor.matmul(out=pt[:, 1, 0:ECH], lhsT=lhs,
                                             rhs=w2ap(c1), start=True,
                                             stop=True)
                        even = (tt * NPAIR + p) % 2 == 0
                        if c1 < NEC:
                            osl = o_sb[:, c0 * ECH:(c1 + 1) * ECH]
                            dst = osl.rearrange("q (two c) -> q two c", two=2)
                            src = pt[:, :, 0:ECH]
                        else:
                            dst = o_sb[:, c0 * ECH:(c0 + 1) * ECH]
                            src = pt[:, 0, 0:ECH]
                        if has_b2:
                            if c1 < NEC:
                                bsl = bb_sb[:, c0 * ECH:(c1 + 1) * ECH]
                                bsl = bsl.rearrange("q (two c) -> q two c",
                                                    two=2)
                            else:
                                bsl = bb_sb[:, c0 * ECH:(c0 + 1) * ECH]
                            nc.vector.tensor_tensor(dst, src, bsl, op=AF.add)
                        elif even:
                            nc.vector.tensor_copy(dst, src)
                        else:
                            nc.scalar.copy(dst, src)
                        if p == 2:
                            nc.sync.dma_start(d_out[trow, 0:3000],
                                              o_sb[:, 0:3000])
                        elif p == 5:
                            nc.gpsimd.dma_start(d_out[trow, 3000:HCOLS],
                                                o_sb[:, 3000:HCOLS])
                    nc.gpsimd.dma_start(d_out[trow, HCOLS:ES],
                                        o_sb[:, HCOLS:ES])

    nc.finalize()
    return nc


_NC_CACHE = {}


def _get_nc(has_b2: bool, dwidths: tuple):
    key = (has_b2, dwidths)
    if key not in _NC_CACHE:
        _NC_CACHE[key] = build_nc(has_b2, dwidths)
    return _NC_CACHE[key]


def prep_core_inputs(inputs):
    """Host-side folds + compaction. Returns (shared, w2s, b2s, meta)."""
    ids = np.asarray(inputs["input_ids"]).astype(np.int64)[:, :L]      # [16, 32]
    pos = np.asarray(inputs["entity_position_ids"])                    # [B, M, L]
    am = np.asarray(inputs["entity_attention_mask"])                   # [B, M]
    embed = np.asarray(inputs["embed"], dtype=np.float32)
    gamma = np.asarray(inputs["ln_gamma"], dtype=np.float32)
    beta = np.asarray(inputs["ln_beta"], dtype=np.float32)
    w1 = np.asarray(inputs["w1"], dtype=np.float32)
    w2 = np.asarray(inputs["w2"], dtype=np.float32)
    b2 = np.asarray(inputs["b2"], dtype=np.float32)

    mrow = (pos != -1)                                                 # [B, M, L]
    active = (am != 0) & mrow.any(-1)                                  # [B, M]

    emb = embed[ids]                                                   # [B, 32, D]
    w1g = w1 * gamma[:, None]                                          # [D, R]
    ew1 = emb @ w1g                                                    # [B, 32, R]
    # 1/D folded: s1 matmul yields mu, gram quadratic form yields E[x^2]
    gram = np.einsum('bxd,byd->bxy', emb, emb) / np.float32(D)         # [B,32,32]
    esum = emb.sum(-1) / np.float32(D)                                 # [B, 32]
    c_row = beta @ w1                                                  # [R]
    u_row = gamma @ w1                                                 # [R]

    # group g holds batches 4g..4g+3 on partitions 32k..32k+32 (k = b-4g);
    # groups occupy consecutive column ranges of variable width.
    tok_lists = []
    for g in range(4):
        toks = []
        for k in range(4):
            b = 4 * g + k
            for j in np.nonzero(active[b])[0]:
                toks.append((k, b, int(j)))
        tok_lists.append(toks)
    n_act = sum(len(t) for t in tok_lists)
    TP = (n_act // 128) * 128          # device tokens; remainder on host
    # clip the flat token list at TP to get device widths
    dwidths, acc = [], 0
    for g in range(4):
        take = max(0, min(len(tok_lists[g]), TP - acc))
        dwidths.append(take)
        acc += take
    dwidths = tuple(dwidths)

    tok_flat = [t for g in range(4) for t in tok_lists[g]]
    tok_idx = np.asarray([b * M + j for (_, b, j) in tok_flat], np.int64)

    PK_EW1, PK_GG, PK_ES = TP, TP + 512, TP + 1024
    PK = TP + 1028
    PKP = ((PK + 15) // 16) * 16
    packed = np.zeros((128, PKP), np.float32)
    for col, (k, b, j) in enumerate(tok_flat[:TP]):
        packed[32 * k:32 * k + 32, col] = mrow[b, j] / np.float32(L)
    for g in range(4):
        for k in range(4):
            b = 4 * g + k
            rows = slice(32 * k, 32 * k + 32)
            packed[rows, PK_EW1 + g * 128:PK_EW1 + (g + 1) * 128] = ew1[b]
            packed[rows, PK_GG + g * 128 + 32 * k:
                   PK_GG + g * 128 + 32 * k + 32] = gram[b]
            packed[rows, PK_ES + g] = esum[b]
    curow = np.concatenate([c_row, u_row]).reshape(1, 2 * R).astype(np.float32)

    shared = {
        "packed": packed.astype(BF16NP),
        "curow": curow,
        "onesr": np.ones((1, 128), np.float32),
    }
    w2s = [np.ascontiguousarray(w2[:, c * ES:(c + 1) * ES]).astype(BF16NP)
           for c in range(N_CORES)]
    b2s = [np.ascontiguousarray(b2[c * ES:(c + 1) * ES].reshape(1, ES))
           for c in range(N_CORES)]
    has_b2 = bool(np.any(b2 != 0.0))
    const_row = (beta @ w1) @ w2 + b2                                  # [E]

    # host-side fp32 rows for the remainder tokens (and the TTd==0 case)
    rem_rows = None
    if n_act > TP:
        rsel = tok_flat[TP:]
        bs = np.asarray([b for (_, b, _) in rsel])
        js = np.asarray([j for (_, _, j) in rsel])
        mr = mrow[bs, js].astype(np.float32) / np.float32(L)           # [nr, 32]
        pooled_r = np.einsum('rx,rxd->rd', mr, emb[bs])                # [nr, D]
        mu = pooled_r.mean(-1, keepdims=True)
        var = ((pooled_r - mu) ** 2).mean(-1, keepdims=True)
        x = (pooled_r - mu) / np.sqrt(var + LN_EPS) * gamma + beta
        rem_rows = (x @ w1) @ w2 + b2                                  # [nr, E]

    meta = {
        "has_b2": has_b2,
        "dwidths": dwidths,
        "TP": TP,
        "tok_idx": tok_idx,
        "const_row": const_row.astype(np.float32),
        "rem_rows": rem_rows,
        "active": active,
    }
    return shared, w2s, b2s, meta


def _bf16_to_f32(a):
    return (a.view(np.uint16).astype(np.uint32) << 16).view(np.float32)


def kernel(**inputs) -> np.ndarray:
    shared, w2s, b2s, meta = prep_core_inputs(inputs)
    TP = meta["TP"]
    full = np.zeros((B * M, E), np.float32)
    tok_idx = meta["tok_idx"]
    if TP > 0:
        nc = _get_nc(meta["has_b2"], meta["dwidths"])
        in_maps = [dict(shared, w2s=w2s[c], b2s=b2s[c])
                   for c in range(N_CORES)]
        res = run_bass_kernel_spmd(nc, in_maps, list(range(N_CORES)))
        buf = np.empty((TP, E), np.float32)
        for c in range(N_CORES):
            blk = np.asarray(res.results[c]["out"])          # bf16 [TP, ES]
            buf[:, c * ES:(c + 1) * ES] = _bf16_to_f32(
                np.ascontiguousarray(blk))
        full[tok_idx[:TP]] = buf
    if meta["rem_rows"] is not None:
        full[tok_idx[TP:]] = meta["rem_rows"]
    cr = meta["const_row"]
    if np.any(cr != 0.0):
        inactive = np.nonzero(~meta["active"].reshape(-1))[0]
        full[inactive] = cr
    return np.ascontiguousarray(full.reshape(B, M, E))
